# revision 12
# baseline (speedup 1.0000x reference)
"""DSAttention (de-stationary attention) TRN2 Bass kernel, v2.

Computes, per (b, h):
    scores = (q @ k^T) * tau_b + delta_b          [L, S]
    scores = where(causal_mask, -1e9, scores)
    A = softmax(scale * scores)
    out = A @ v                                    [L, D]

Strategy: batch*head parallel over 8 cores (4 (b,h) pairs per core).
Per (b,h), everything is computed in "transposed score" space:
    X_T[s, l] = sum_e KT[e, s] * QT[e, l]     (QT pre-scaled by a'*scale*tau
                                               on host, a' = 128/ln2)
    p = exp-ish(X_T)                          (delta via an augmentation row)
    OutT[d, l] = sum_s V'[s, d] * p[s, l]     (V' has a ones column -> row 64
                                               of OutT is the softmax denom)
host divides + transposes the raw numerator/denominator output.

v2 changes vs v1 (101.6us):
  * all matmuls in bf16 (same PE rate as f32r but no N<256 4x penalty and
    no need to pad the contraction to 128 partitions or extend short
    pieces: causal pieces are packed contiguously into PSUM group tiles)
  * the exp is split between the Activation engine (exact exp, out bf16)
    and the DVE (Schraudolph-style exp2 bit trick: the QK matmul output is
    already a'-scaled, so bf16 bits = int16(X + b0); one tensor_scalar
    add with int16-converting output per element), removing the single-
    engine activation bottleneck (~76us on ACT in v1)
  * depth-2 software pipelining of the PE stream (QK(g) ... AV(g-2)) so
    the exp latency hides behind matmul work
  * output evacuation PSUM->SBUF alternates ACT/DVE (DMA can't read PSUM)
"""

import math

import numpy as np
import ml_dtypes

import bass_rust
import concourse.bass as bass
import concourse.mybir as mybir
import concourse.tile as tile
from concourse.bass_utils import run_bass_kernel_spmd

B, L, S, H, E, D = 2, 2048, 2048, 16, 64, 64
NCORES = 8
BH = B * H                      # 32 (b,h) pairs
BH_PER_CORE = BH // NCORES      # 4
SCALE = 1.0 / math.sqrt(E)
APRIME = 128.0 / math.log(2.0)  # exp(x) = 2^(a'*x / 128); folded into q/delta
B0 = 16248.65                   # Schraudolph bf16 bias, gmean-centered
LOG2_OVER_128 = math.log(2.0) / 128.0

F32 = mybir.dt.float32
BF16 = mybir.dt.bfloat16
I16 = mybir.dt.int16

VP_COLS = (S // 128) * (D + 1)  # 1040
NP_BF16 = ml_dtypes.bfloat16


class _SplitDrainTileContext(tile.TileContext):
    """This walrus build rejects instructions carrying more than one sem
    wait; the kernel-tail drain aggregates one wait per active processor.
    Split them across a chain of drains on the same engine."""

    def _drain_and_barrier(self, tick_clock, wait_clock):
        nc = self.nc
        drain_inst = nc.sync.drain()
        wait_clock.add_sem_waits(
            drain_inst.ins, bass_rust.ScopedClock({None: tick_clock.global_clock})
        )
        si = drain_inst.ins.sync_info
        waits = list(si.on_wait) if si is not None and si.on_wait else []
        if len(waits) > 1:
            si.on_wait = waits[:1]
            for w in waits[1:]:
                d2 = nc.sync.drain()
                d2.ins.sync_info = bass_rust.SyncInfo(on_wait=[w], on_update=[])
        nc.all_engine_barrier()
        popped = nc._tile_sem_poison_stack.pop()
        assert popped is self._sem_poison
        nc.clear_and_free_semaphores(list(self.sems.allocated().values()))
        nc.all_engine_barrier()


def _legalize_waits(nc, max_waits=1):
    """This walrus build rejects instructions with more than `max_waits`
    sem waits. Spill extras onto same-engine NoOps inserted just before
    the offending instruction (same-engine program order preserves the
    wait semantics)."""
    for f in nc.m.functions:
        for bb in f.blocks:
            insts = bb.instructions
            for idx in range(len(insts) - 1, -1, -1):
                inst = insts[idx]
                si = getattr(inst, "sync_info", None)
                if si is None or not si.on_wait:
                    continue
                ow = list(si.on_wait)
                sem = [w for w in ow if w.sync_type == "semaphore"]
                other = [w for w in ow if w.sync_type != "semaphore"]
                budget = max(0, max_waits - len(other))
                if len(sem) <= budget:
                    continue
                keep, spill = sem[:budget], sem[budget:]
                si.on_wait = other + keep
                for w in reversed(spill):
                    n = mybir.InstNoOp(name=f"W-{nc.next_id()}", ins=[], outs=[])
                    n.engine = inst.engine
                    n.sync_info = bass_rust.SyncInfo(on_wait=[w], on_update=[])
                    nc.register_instruction(n, overwrite=True)
                    insts.insert(idx, n)


def _core_groups():
    """Static schedule: the packed causal piece/group structure for one core.

    Per (bh, qr) the valid strips are j = 0..4qr+3 with piece widths
    512 (j <= 4qr), 384, 256, 128. Pieces pack contiguously into PSUM
    group tiles (<= 1024 cols, never crossing a 512-col bank boundary):
      pairs  [512, 512] x 2qr   (alternating ACT/DVE)
      A      [512, 384]         (ACT)
      B      [256, 128]         (DVE)
    Each piece: (j, off, ln); derived rel = max(0,128j-l0), diag = j>=4qr.
    """
    groups = []
    for i in range(BH_PER_CORE):
        # bh3 runs its quarters in reverse so the final pipeline-drain
        # chain ends on qr0's small groups (shorter serial tail)
        qrs = (3, 2, 1, 0) if i == BH_PER_CORE - 1 else (0, 1, 2, 3)
        for qr in qrs:
            qgroups = []
            for t in range(2 * qr):
                qgroups.append(dict(
                    engine='act' if t % 2 == 0 else 'dve',
                    pieces=[(2 * t, 0, 512), (2 * t + 1, 512, 512)]))
            qgroups.append(dict(
                engine='act', pieces=[(4 * qr, 0, 512), (4 * qr + 1, 512, 384)]))
            qgroups.append(dict(
                engine='dve', pieces=[(4 * qr + 2, 0, 256), (4 * qr + 3, 256, 128)]))
            for gi, g in enumerate(qgroups):
                g.update(i=i, qr=qr, last=(gi == len(qgroups) - 1))
                groups.append(g)
    return groups


def _build_program():
    nc = bass.Bass("TRN2", target_bir_lowering=False, debug=False)
    # q: rows 0:64 = (a'*scale*tau)*q^T, row 64 = 1.0
    # k: rows 0:64 = k^T,                row 64 = (a'*scale)*delta
    # v: col 65*j + c = V'[128j+p, c], V' = [v | ones]
    q_d = nc.declare_dram_parameter("q", [BH_PER_CORE, E + 1, L], BF16, isOutput=False)
    k_d = nc.declare_dram_parameter("k", [BH_PER_CORE, E + 1, S], BF16, isOutput=False)
    v_d = nc.declare_dram_parameter("v", [BH_PER_CORE, 128, VP_COLS], BF16, isOutput=False)
    # output stays transposed: [bh, quarter, d, l_rel]; row d == D is the
    # softmax denominator; the host divides + transposes.
    o_d = nc.declare_dram_parameter("o", [BH_PER_CORE, 4, D + 1, 512], F32, isOutput=True)

    groups = _core_groups()

    with _SplitDrainTileContext(nc) as tc:
        with (
            tc.tile_pool(name="qin", bufs=1) as q_pool,
            tc.tile_pool(name="p", bufs=4) as p_pool,
            tc.tile_pool(name="osb", bufs=3) as osb_pool,
            tc.tile_pool(name="xt_ps", bufs=3, space="PSUM") as xt_pool,
            tc.tile_pool(name="out_ps", bufs=2, space="PSUM") as ot_pool,
        ):
            qts, kts, vts = [], [], []
            for i in range(BH_PER_CORE):
                qts.append(q_pool.tile([E + 1, L], BF16, name=f"q{i}", tag=f"q{i}"))
                kts.append(q_pool.tile([E + 1, S], BF16, name=f"k{i}", tag=f"k{i}"))
                vts.append(q_pool.tile([128, VP_COLS], BF16, name=f"v{i}", tag=f"v{i}"))
            # bh0's first pieces are on the critical path: issue their DMAs
            # from three different engines in parallel (HWDGE issue is
            # ~0.6us each; SP alone would serialize 0.6us per dma_start)
            # and partition-split them across two queues each.
            nc.scalar.dma_start(out=qts[0][0:33, 0:512], in_=q_d[0, 0:33, 0:512])
            nc.scalar.dma_start(out=qts[0][33:65, 0:512], in_=q_d[0, 33:65, 0:512])
            nc.sync.dma_start(out=kts[0][0:33, 0:512], in_=k_d[0, 0:33, 0:512])
            nc.sync.dma_start(out=kts[0][33:65, 0:512], in_=k_d[0, 33:65, 0:512])
            nc.sync.dma_start(out=vts[0][0:64, 0:260], in_=v_d[0, 0:64, 0:260])
            nc.sync.dma_start(out=vts[0][64:128, 0:260], in_=v_d[0, 64:128, 0:260])
            nc.sync.dma_start(out=qts[0][:, 512:1024], in_=q_d[0, :, 512:1024])
            nc.sync.dma_start(out=kts[0][:, 512:1024], in_=k_d[0, :, 512:1024])
            nc.sync.dma_start(out=vts[0][:, 260:520], in_=v_d[0, :, 260:520])
            nc.sync.dma_start(out=qts[0][:, 1024:2048], in_=q_d[0, :, 1024:2048])
            nc.sync.dma_start(out=kts[0][:, 1024:2048], in_=k_d[0, :, 1024:2048])
            nc.sync.dma_start(out=vts[0][:, 520:1040], in_=v_d[0, :, 520:1040])
            # bh1 rides the SP queue up front; bh2/bh3 go out the Activation
            # engine's queue (deferred into the group loop so ACT's
            # sequencer isn't blocked ahead of its first exp). bh3 is
            # consumed qr3-first, so its k/v ship before q.
            i = 1
            nc.sync.dma_start(out=qts[i][:, 0:1024], in_=q_d[i, :, 0:1024])
            nc.sync.dma_start(out=kts[i][:, 0:1024], in_=k_d[i, :, 0:1024])
            nc.sync.dma_start(out=vts[i][:, 0:520], in_=v_d[i, :, 0:520])
            nc.sync.dma_start(out=qts[i][:, 1024:2048], in_=q_d[i, :, 1024:2048])
            nc.sync.dma_start(out=kts[i][:, 1024:2048], in_=k_d[i, :, 1024:2048])
            nc.sync.dma_start(out=vts[i][:, 520:1040], in_=v_d[i, :, 520:1040])
            deferred = []
            i = 2
            for (t, lo, hi) in (("q", 0, 1024), ("k", 0, 1024), ("v", 0, 520),
                                ("q", 1024, 2048), ("k", 1024, 2048), ("v", 520, 1040)):
                deferred.append((t, i, lo, hi))
            i = 3
            for (t, lo, hi) in (("k", 0, 1024), ("k", 1024, 2048), ("v", 0, 520),
                                ("v", 520, 1040), ("q", 1024, 2048), ("q", 0, 1024)):
                deferred.append((t, i, lo, hi))

            ots = {}          # (i, qr) -> out PSUM tile
            evac_n = [0]

            def emit_av(g):
                i, qr = g['i'], g['qr']
                l0 = 512 * qr
                if (i, qr) not in ots:
                    ots[(i, qr)] = ot_pool.tile([D + 1, 512], F32, name="ot", tag="ot")
                ot = ots[(i, qr)]
                for (j, off, ln) in g['pieces']:
                    rel = max(0, 128 * j - l0)
                    nc.tensor.matmul(
                        ot[:, rel:rel + ln],
                        lhsT=vts[i][:, (D + 1) * j:(D + 1) * (j + 1)],
                        rhs=g['p'][:, off:off + ln].bitcast(BF16),
                        start=(j == 0), stop=(j == 4 * qr + 3),
                    )
                if g['last']:
                    ot_sb = osb_pool.tile([D + 1, 512], F32)
                    if evac_n[0] % 2 == 0:
                        nc.scalar.activation(
                            ot_sb, ot, mybir.ActivationFunctionType.Copy)
                    else:
                        nc.vector.tensor_copy(ot_sb, ot)
                    evac_n[0] += 1
                    # each engine owns ONE hw DMA queue; balance bytes:
                    # bh0/1 outputs ride SP (its input backlog clears
                    # early), bh2/3 ride ACT. Final output splits across
                    # both queues to halve the post-evac dead time.
                    if evac_n[0] == 4 * BH_PER_CORE:
                        nc.scalar.dma_start(
                            out=o_d[i, qr, :, 0:256], in_=ot_sb[:, 0:256])
                        nc.sync.dma_start(
                            out=o_d[i, qr, :, 256:512], in_=ot_sb[:, 256:512])
                    elif i < 2:
                        nc.sync.dma_start(out=o_d[i, qr], in_=ot_sb)
                    else:
                        nc.scalar.dma_start(out=o_d[i, qr], in_=ot_sb)

            tiles_of = {"q": qts, "k": kts, "v": vts}
            dram_of = {"q": q_d, "k": k_d, "v": v_d}
            for gidx, g in enumerate(groups):
                # drip one deferred bh2/bh3 input DMA per ACT-assigned group
                # (bh2 lands by ~group 12, bh3 by ~group 26; both are
                # consumed much later)
                if deferred and g['engine'] == 'act':
                    t, di, lo, hi = deferred.pop(0)
                    nc.scalar.dma_start(out=tiles_of[t][di][:, lo:hi],
                                        in_=dram_of[t][di, :, lo:hi])
                i, qr = g['i'], g['qr']
                l0 = 512 * qr
                width = sum(ln for (_, _, ln) in g['pieces'])
                xt = xt_pool.tile([128, 1024], F32)
                for (j, off, ln) in g['pieces']:
                    ls = max(l0, 128 * j)
                    nc.tensor.matmul(
                        xt[:, off:off + ln],
                        lhsT=kts[i][:, 128 * j:128 * (j + 1)],
                        rhs=qts[i][:, ls:ls + ln],
                        start=True, stop=True,
                    )
                p = p_pool.tile([128, 1024], BF16)
                g['p'] = p
                if g['engine'] == 'act':
                    nc.scalar.activation(
                        p[:, 0:width], xt[:, 0:width],
                        mybir.ActivationFunctionType.Exp,
                        scale=LOG2_OVER_128,
                    )
                else:
                    nc.vector.tensor_scalar(
                        p[:, 0:width].bitcast(I16), xt[:, 0:width],
                        float(B0), None, mybir.AluOpType.add,
                    )
                for (j, off, ln) in g['pieces']:
                    if 128 * j >= l0:
                        # diagonal block: zero p where s > l
                        # (keep where (l - s) >= 0)
                        nc.gpsimd.affine_select(
                            out=p[:, off:off + 128],
                            in_=p[:, off:off + 128],
                            compare_op=mybir.AluOpType.is_ge, fill=0.0,
                            base=0, channel_multiplier=-1,
                            pattern=[[1, 128]],
                        )
                # depth-2 software pipeline: PE stream is QK(g), AV(g-2)
                if gidx >= 2:
                    emit_av(groups[gidx - 2])
            emit_av(groups[-2])
            emit_av(groups[-1])
    _legalize_waits(nc)
    return nc


_PROGRAM = None


def _get_program():
    global _PROGRAM
    if _PROGRAM is None:
        _PROGRAM = _build_program()
    return _PROGRAM


def _prepare_inputs(q, k, v, tau, delta):
    """Pack full inputs into per-core bf16 device layouts."""
    qs = (q.astype(np.float64)
          * (SCALE * APRIME * tau.astype(np.float64))[:, 0, None, None, None]
          ).astype(np.float32)
    qt = qs.transpose(0, 2, 3, 1).reshape(BH, E, L)
    kt = k.transpose(0, 2, 3, 1).reshape(BH, E, S)
    dsc = (SCALE * APRIME * delta).astype(np.float32)    # [B, S]
    xq = np.concatenate([qt, np.ones((BH, 1, L), np.float32)], 1)
    xk = np.concatenate([kt, np.repeat(dsc, H, axis=0)[:, None, :]], 1)
    vt = v.transpose(0, 2, 1, 3).reshape(BH, S, D)
    vp = np.concatenate([vt, np.ones((BH, S, 1), np.float32)], axis=2)
    vp = (vp.reshape(BH, S // 128, 128, D + 1)
          .transpose(0, 2, 1, 3).reshape(BH, 128, VP_COLS))
    return (np.ascontiguousarray(xq).astype(NP_BF16),
            np.ascontiguousarray(xk).astype(NP_BF16),
            np.ascontiguousarray(vp).astype(NP_BF16))


def _numpy_fallback(q, k, v, att_mask, tau, delta):
    out = np.empty((B, L, H, D), np.float32)
    mask = att_mask[:, 0]  # [B, L, S]
    for b in range(B):
        for h in range(H):
            s = (q[b, :, h, :] @ k[b, :, h, :].T) * tau[b, 0] + delta[b][None, :]
            s = np.where(mask[b], -1e9, s).astype(np.float32)
            s = SCALE * s
            s = s - s.max(axis=-1, keepdims=True)
            e = np.exp(s)
            a = e / e.sum(axis=-1, keepdims=True)
            out[b, :, h, :] = a @ v[b, :, h, :]
    return out


def kernel(q, k, v, att_mask, tau, delta):
    q = np.asarray(q, np.float32)
    k = np.asarray(k, np.float32)
    v = np.asarray(v, np.float32)
    tau = np.asarray(tau, np.float32)
    delta = np.asarray(delta, np.float32)
    att_mask = np.asarray(att_mask)

    causal = np.triu(np.ones((L, S), bool), k=1)
    if not all(np.array_equal(att_mask[b, 0], causal) for b in range(B)):
        return _numpy_fallback(q, k, v, att_mask, tau, delta)

    xq, xk, vp = _prepare_inputs(q, k, v, tau, delta)
    nc = _get_program()
    in_maps = [
        {
            "q": np.ascontiguousarray(xq[c * BH_PER_CORE:(c + 1) * BH_PER_CORE]),
            "k": np.ascontiguousarray(xk[c * BH_PER_CORE:(c + 1) * BH_PER_CORE]),
            "v": np.ascontiguousarray(vp[c * BH_PER_CORE:(c + 1) * BH_PER_CORE]),
        }
        for c in range(NCORES)
    ]
    res = run_bass_kernel_spmd(nc, in_maps, list(range(NCORES))).results

    out = np.empty((B, L, H, D), np.float32)
    for c in range(NCORES):
        o = res[c]["o"]  # [4, 4, D+1, 512]: raw numerators + denominator row
        norm = o[:, :, 0:D, :] / o[:, :, D:D + 1, :]
        for i in range(BH_PER_CORE):
            bh = c * BH_PER_CORE + i
            out[bh // H, :, bh % H, :] = norm[i].transpose(0, 2, 1).reshape(L, D)
    return out


# revision 16
# speedup vs baseline: 1.0554x; 1.0554x over previous
"""DSAttention (de-stationary attention) TRN2 Bass kernel, v2.

Computes, per (b, h):
    scores = (q @ k^T) * tau_b + delta_b          [L, S]
    scores = where(causal_mask, -1e9, scores)
    A = softmax(scale * scores)
    out = A @ v                                    [L, D]

Strategy: batch*head parallel over 8 cores (4 (b,h) pairs per core).
Per (b,h), everything is computed in "transposed score" space:
    X_T[s, l] = sum_e KT[e, s] * QT[e, l]     (QT pre-scaled by a'*scale*tau
                                               on host, a' = 128/ln2)
    p = exp-ish(X_T)                          (delta via an augmentation row)
    OutT[d, l] = sum_s V'[s, d] * p[s, l]     (V' has a ones column -> row 64
                                               of OutT is the softmax denom)
host divides + transposes the raw numerator/denominator output.

v2 changes vs v1 (101.6us):
  * all matmuls in bf16 (same PE rate as f32r but no N<256 4x penalty and
    no need to pad the contraction to 128 partitions or extend short
    pieces: causal pieces are packed contiguously into PSUM group tiles)
  * the exp is split between the Activation engine (exact exp, out bf16)
    and the DVE (Schraudolph-style exp2 bit trick: the QK matmul output is
    already a'-scaled, so bf16 bits = int16(X + b0); one tensor_scalar
    add with int16-converting output per element), removing the single-
    engine activation bottleneck (~76us on ACT in v1)
  * depth-2 software pipelining of the PE stream (QK(g) ... AV(g-2)) so
    the exp latency hides behind matmul work
  * output evacuation PSUM->SBUF alternates ACT/DVE (DMA can't read PSUM)
"""

import math

import numpy as np
import ml_dtypes

import bass_rust
import concourse.bass as bass
import concourse.mybir as mybir
import concourse.tile as tile
from concourse.bass_utils import run_bass_kernel_spmd

B, L, S, H, E, D = 2, 2048, 2048, 16, 64, 64
NCORES = 8
BH = B * H                      # 32 (b,h) pairs
BH_PER_CORE = BH // NCORES      # 4
SCALE = 1.0 / math.sqrt(E)
APRIME = 128.0 / math.log(2.0)  # exp(x) = 2^(a'*x / 128); folded into q/delta
B0 = 16248.65                   # Schraudolph bf16 bias, gmean-centered
LOG2_OVER_128 = math.log(2.0) / 128.0

F32 = mybir.dt.float32
BF16 = mybir.dt.bfloat16
I16 = mybir.dt.int16

VP_COLS = (S // 128) * (D + 1)  # 1040
NP_BF16 = ml_dtypes.bfloat16


class _SplitDrainTileContext(tile.TileContext):
    """This walrus build rejects instructions carrying more than one sem
    wait; the kernel-tail drain aggregates one wait per active processor.
    Split them across a chain of drains on the same engine."""

    def _drain_and_barrier(self, tick_clock, wait_clock):
        nc = self.nc
        drain_inst = nc.sync.drain()
        wait_clock.add_sem_waits(
            drain_inst.ins, bass_rust.ScopedClock({None: tick_clock.global_clock})
        )
        si = drain_inst.ins.sync_info
        waits = list(si.on_wait) if si is not None and si.on_wait else []
        if len(waits) > 1:
            si.on_wait = waits[:1]
            for w in waits[1:]:
                d2 = nc.sync.drain()
                d2.ins.sync_info = bass_rust.SyncInfo(on_wait=[w], on_update=[])
        nc.all_engine_barrier()
        popped = nc._tile_sem_poison_stack.pop()
        assert popped is self._sem_poison
        nc.clear_and_free_semaphores(list(self.sems.allocated().values()))
        nc.all_engine_barrier()


def _legalize_waits(nc, max_waits=1):
    """This walrus build rejects instructions with more than `max_waits`
    sem waits. Spill extras onto same-engine NoOps inserted just before
    the offending instruction (same-engine program order preserves the
    wait semantics)."""
    for f in nc.m.functions:
        for bb in f.blocks:
            insts = bb.instructions
            for idx in range(len(insts) - 1, -1, -1):
                inst = insts[idx]
                si = getattr(inst, "sync_info", None)
                if si is None or not si.on_wait:
                    continue
                ow = list(si.on_wait)
                sem = [w for w in ow if w.sync_type == "semaphore"]
                other = [w for w in ow if w.sync_type != "semaphore"]
                budget = max(0, max_waits - len(other))
                if len(sem) <= budget:
                    continue
                keep, spill = sem[:budget], sem[budget:]
                si.on_wait = other + keep
                for w in reversed(spill):
                    n = mybir.InstNoOp(name=f"W-{nc.next_id()}", ins=[], outs=[])
                    n.engine = inst.engine
                    n.sync_info = bass_rust.SyncInfo(on_wait=[w], on_update=[])
                    nc.register_instruction(n, overwrite=True)
                    insts.insert(idx, n)


def _core_groups():
    """Static schedule: the packed causal piece/group structure for one core.

    Per (bh, qr) the valid strips are j = 0..4qr+3 with piece widths
    512 (j <= 4qr), 384, 256, 128. Pieces pack contiguously into PSUM
    group tiles (<= 1024 cols, never crossing a 512-col bank boundary):
      pairs  [512, 512] x 2qr   (alternating ACT/DVE)
      A      [512, 384]         (ACT)
      B      [256, 128]         (DVE)
    Each piece: (j, off, ln); derived rel = max(0,128j-l0), diag = j>=4qr.
    """
    groups = []
    for i in range(BH_PER_CORE):
        for qr in range(4):
            qgroups = []
            for t in range(2 * qr):
                qgroups.append(dict(
                    engine='act' if t % 2 == 0 else 'dve',
                    pieces=[(2 * t, 0, 512), (2 * t + 1, 512, 512)]))
            qgroups.append(dict(
                engine='act', pieces=[(4 * qr, 0, 512), (4 * qr + 1, 512, 384)]))
            qgroups.append(dict(
                engine='dve', pieces=[(4 * qr + 2, 0, 256), (4 * qr + 3, 256, 128)]))
            for gi, g in enumerate(qgroups):
                g.update(i=i, qr=qr, last=(gi == len(qgroups) - 1))
                groups.append(g)
    return groups


def _build_program():
    nc = bass.Bass("TRN2", target_bir_lowering=False, debug=False)
    # q: rows 0:64 = (a'*scale*tau)*q^T, row 64 = 1.0
    # k: rows 0:64 = k^T,                row 64 = (a'*scale)*delta
    # v: col 65*j + c = V'[128j+p, c], V' = [v | ones]
    q_d = nc.declare_dram_parameter("q", [BH_PER_CORE, E + 1, L], BF16, isOutput=False)
    k_d = nc.declare_dram_parameter("k", [BH_PER_CORE, E + 1, S], BF16, isOutput=False)
    v_d = nc.declare_dram_parameter("v", [BH_PER_CORE, 128, VP_COLS], BF16, isOutput=False)
    # output stays transposed: [bh, quarter, d, l_rel]; row d == D is the
    # softmax denominator; the host divides + transposes.
    o_d = nc.declare_dram_parameter("o", [BH_PER_CORE, 4, D + 1, 512], F32, isOutput=True)

    groups = _core_groups()

    with _SplitDrainTileContext(nc) as tc:
        with (
            tc.tile_pool(name="qin", bufs=1) as q_pool,
            tc.tile_pool(name="p", bufs=4) as p_pool,
            tc.tile_pool(name="osb", bufs=3) as osb_pool,
            tc.tile_pool(name="xt_ps", bufs=3, space="PSUM") as xt_pool,
            tc.tile_pool(name="out_ps", bufs=2, space="PSUM") as ot_pool,
        ):
            qts, kts, vts = [], [], []
            for i in range(BH_PER_CORE):
                qts.append(q_pool.tile([E + 1, L], BF16, name=f"q{i}", tag=f"q{i}"))
                kts.append(q_pool.tile([E + 1, S], BF16, name=f"k{i}", tag=f"k{i}"))
                vts.append(q_pool.tile([128, VP_COLS], BF16, name=f"v{i}", tag=f"v{i}"))
            # bh0's first pieces are on the critical path: issue their DMAs
            # from three different engines in parallel (HWDGE issue is
            # ~0.6us each; SP alone would serialize 0.6us per dma_start)
            # and partition-split them across two queues each.
            nc.scalar.dma_start(out=qts[0][0:33, 0:512], in_=q_d[0, 0:33, 0:512])
            nc.scalar.dma_start(out=qts[0][33:65, 0:512], in_=q_d[0, 33:65, 0:512])
            nc.sync.dma_start(out=kts[0][0:33, 0:512], in_=k_d[0, 0:33, 0:512])
            nc.sync.dma_start(out=kts[0][33:65, 0:512], in_=k_d[0, 33:65, 0:512])
            # all v tensors ride the gpsimd (Pool) DMA queue: Pool's SWDGE
            # issue is slow (~1us each) but Pool is idle until its first
            # affine_select (~12.5us), and this takes 1.1MB off the SP
            # queue, whose backlog otherwise starves bh3's last QK groups.
            nc.gpsimd.dma_start(out=vts[0][0:64, 0:260], in_=v_d[0, 0:64, 0:260])
            nc.gpsimd.dma_start(out=vts[0][64:128, 0:260], in_=v_d[0, 64:128, 0:260])
            nc.gpsimd.dma_start(out=vts[0][:, 260:520], in_=v_d[0, :, 260:520])
            nc.gpsimd.dma_start(out=vts[0][:, 520:1040], in_=v_d[0, :, 520:1040])
            nc.sync.dma_start(out=qts[0][:, 512:1024], in_=q_d[0, :, 512:1024])
            nc.sync.dma_start(out=kts[0][:, 512:1024], in_=k_d[0, :, 512:1024])
            nc.sync.dma_start(out=qts[0][:, 1024:2048], in_=q_d[0, :, 1024:2048])
            nc.sync.dma_start(out=kts[0][:, 1024:2048], in_=k_d[0, :, 1024:2048])
            for i in range(1, BH_PER_CORE):
                nc.gpsimd.dma_start(out=vts[i][:, 0:520], in_=v_d[i, :, 0:520])
                nc.sync.dma_start(out=qts[i][:, 0:1024], in_=q_d[i, :, 0:1024])
                nc.sync.dma_start(out=kts[i][:, 0:1024], in_=k_d[i, :, 0:1024])
                nc.gpsimd.dma_start(out=vts[i][:, 520:1040], in_=v_d[i, :, 520:1040])
                nc.sync.dma_start(out=qts[i][:, 1024:2048], in_=q_d[i, :, 1024:2048])
                nc.sync.dma_start(out=kts[i][:, 1024:2048], in_=k_d[i, :, 1024:2048])

            ots = {}          # (i, qr) -> out PSUM tile
            evac_n = [0]

            def emit_av(g):
                i, qr = g['i'], g['qr']
                l0 = 512 * qr
                if (i, qr) not in ots:
                    ots[(i, qr)] = ot_pool.tile([D + 1, 512], F32, name="ot", tag="ot")
                ot = ots[(i, qr)]
                for (j, off, ln) in g['pieces']:
                    rel = max(0, 128 * j - l0)
                    nc.tensor.matmul(
                        ot[:, rel:rel + ln],
                        lhsT=vts[i][:, (D + 1) * j:(D + 1) * (j + 1)],
                        rhs=g['p'][:, off:off + ln].bitcast(BF16),
                        start=(j == 0), stop=(j == 4 * qr + 3),
                    )
                if g['last']:
                    ot_sb = osb_pool.tile([D + 1, 512], F32)
                    if evac_n[0] % 2 == 0:
                        nc.scalar.activation(
                            ot_sb, ot, mybir.ActivationFunctionType.Copy)
                    else:
                        nc.vector.tensor_copy(ot_sb, ot)
                    evac_n[0] += 1
                    # outputs go out the Activation engine's DMA queue (the
                    # SP queue carries the q/k input stream; each engine
                    # owns ONE hw queue). The final output splits 4 ways
                    # across both queues to minimize post-evac dead time.
                    if evac_n[0] == 4 * BH_PER_CORE:
                        nc.scalar.dma_start(
                            out=o_d[i, qr, :, 0:128], in_=ot_sb[:, 0:128])
                        nc.sync.dma_start(
                            out=o_d[i, qr, :, 128:256], in_=ot_sb[:, 128:256])
                        nc.scalar.dma_start(
                            out=o_d[i, qr, :, 256:384], in_=ot_sb[:, 256:384])
                        nc.sync.dma_start(
                            out=o_d[i, qr, :, 384:512], in_=ot_sb[:, 384:512])
                    else:
                        nc.scalar.dma_start(out=o_d[i, qr], in_=ot_sb)

            for gidx, g in enumerate(groups):
                i, qr = g['i'], g['qr']
                l0 = 512 * qr
                width = sum(ln for (_, _, ln) in g['pieces'])
                xt = xt_pool.tile([128, 1024], F32)
                for (j, off, ln) in g['pieces']:
                    ls = max(l0, 128 * j)
                    nc.tensor.matmul(
                        xt[:, off:off + ln],
                        lhsT=kts[i][:, 128 * j:128 * (j + 1)],
                        rhs=qts[i][:, ls:ls + ln],
                        start=True, stop=True,
                    )
                p = p_pool.tile([128, 1024], BF16)
                g['p'] = p
                if g['engine'] == 'act':
                    nc.scalar.activation(
                        p[:, 0:width], xt[:, 0:width],
                        mybir.ActivationFunctionType.Exp,
                        scale=LOG2_OVER_128,
                    )
                else:
                    nc.vector.tensor_scalar(
                        p[:, 0:width].bitcast(I16), xt[:, 0:width],
                        float(B0), None, mybir.AluOpType.add,
                    )
                for (j, off, ln) in g['pieces']:
                    if 128 * j >= l0:
                        # diagonal block: zero p where s > l
                        # (keep where (l - s) >= 0)
                        nc.gpsimd.affine_select(
                            out=p[:, off:off + 128],
                            in_=p[:, off:off + 128],
                            compare_op=mybir.AluOpType.is_ge, fill=0.0,
                            base=0, channel_multiplier=-1,
                            pattern=[[1, 128]],
                        )
                # depth-2 software pipeline: PE stream is QK(g), AV(g-2)
                if gidx >= 2:
                    emit_av(groups[gidx - 2])
            emit_av(groups[-2])
            emit_av(groups[-1])
    _legalize_waits(nc)
    return nc


_PROGRAM = None


def _get_program():
    global _PROGRAM
    if _PROGRAM is None:
        _PROGRAM = _build_program()
    return _PROGRAM


def _prepare_inputs(q, k, v, tau, delta):
    """Pack full inputs into per-core bf16 device layouts."""
    qs = (q.astype(np.float64)
          * (SCALE * APRIME * tau.astype(np.float64))[:, 0, None, None, None]
          ).astype(np.float32)
    qt = qs.transpose(0, 2, 3, 1).reshape(BH, E, L)
    kt = k.transpose(0, 2, 3, 1).reshape(BH, E, S)
    dsc = (SCALE * APRIME * delta).astype(np.float32)    # [B, S]
    xq = np.concatenate([qt, np.ones((BH, 1, L), np.float32)], 1)
    xk = np.concatenate([kt, np.repeat(dsc, H, axis=0)[:, None, :]], 1)
    vt = v.transpose(0, 2, 1, 3).reshape(BH, S, D)
    vp = np.concatenate([vt, np.ones((BH, S, 1), np.float32)], axis=2)
    vp = (vp.reshape(BH, S // 128, 128, D + 1)
          .transpose(0, 2, 1, 3).reshape(BH, 128, VP_COLS))
    return (np.ascontiguousarray(xq).astype(NP_BF16),
            np.ascontiguousarray(xk).astype(NP_BF16),
            np.ascontiguousarray(vp).astype(NP_BF16))


def _numpy_fallback(q, k, v, att_mask, tau, delta):
    out = np.empty((B, L, H, D), np.float32)
    mask = att_mask[:, 0]  # [B, L, S]
    for b in range(B):
        for h in range(H):
            s = (q[b, :, h, :] @ k[b, :, h, :].T) * tau[b, 0] + delta[b][None, :]
            s = np.where(mask[b], -1e9, s).astype(np.float32)
            s = SCALE * s
            s = s - s.max(axis=-1, keepdims=True)
            e = np.exp(s)
            a = e / e.sum(axis=-1, keepdims=True)
            out[b, :, h, :] = a @ v[b, :, h, :]
    return out


def kernel(q, k, v, att_mask, tau, delta):
    q = np.asarray(q, np.float32)
    k = np.asarray(k, np.float32)
    v = np.asarray(v, np.float32)
    tau = np.asarray(tau, np.float32)
    delta = np.asarray(delta, np.float32)
    att_mask = np.asarray(att_mask)

    causal = np.triu(np.ones((L, S), bool), k=1)
    if not all(np.array_equal(att_mask[b, 0], causal) for b in range(B)):
        return _numpy_fallback(q, k, v, att_mask, tau, delta)

    xq, xk, vp = _prepare_inputs(q, k, v, tau, delta)
    nc = _get_program()
    in_maps = [
        {
            "q": np.ascontiguousarray(xq[c * BH_PER_CORE:(c + 1) * BH_PER_CORE]),
            "k": np.ascontiguousarray(xk[c * BH_PER_CORE:(c + 1) * BH_PER_CORE]),
            "v": np.ascontiguousarray(vp[c * BH_PER_CORE:(c + 1) * BH_PER_CORE]),
        }
        for c in range(NCORES)
    ]
    res = run_bass_kernel_spmd(nc, in_maps, list(range(NCORES))).results

    out = np.empty((B, L, H, D), np.float32)
    for c in range(NCORES):
        o = res[c]["o"]  # [4, 4, D+1, 512]: raw numerators + denominator row
        norm = o[:, :, 0:D, :] / o[:, :, D:D + 1, :]
        for i in range(BH_PER_CORE):
            bh = c * BH_PER_CORE + i
            out[bh // H, :, bh % H, :] = norm[i].transpose(0, 2, 1).reshape(L, D)
    return out


# revision 19
# speedup vs baseline: 1.0728x; 1.0165x over previous
"""DSAttention (de-stationary attention) TRN2 Bass kernel, v2.

Computes, per (b, h):
    scores = (q @ k^T) * tau_b + delta_b          [L, S]
    scores = where(causal_mask, -1e9, scores)
    A = softmax(scale * scores)
    out = A @ v                                    [L, D]

Strategy: batch*head parallel over 8 cores (4 (b,h) pairs per core).
Per (b,h), everything is computed in "transposed score" space:
    X_T[s, l] = sum_e KT[e, s] * QT[e, l]     (QT pre-scaled by a'*scale*tau
                                               on host, a' = 128/ln2)
    p = exp-ish(X_T)                          (delta via an augmentation row)
    OutT[d, l] = sum_s V'[s, d] * p[s, l]     (V' has a ones column -> row 64
                                               of OutT is the softmax denom)
host divides + transposes the raw numerator/denominator output.

v2 changes vs v1 (101.6us):
  * all matmuls in bf16 (same PE rate as f32r but no N<256 4x penalty and
    no need to pad the contraction to 128 partitions or extend short
    pieces: causal pieces are packed contiguously into PSUM group tiles)
  * the exp is split between the Activation engine (exact exp, out bf16)
    and the DVE (Schraudolph-style exp2 bit trick: the QK matmul output is
    already a'-scaled, so bf16 bits = int16(X + b0); one tensor_scalar
    add with int16-converting output per element), removing the single-
    engine activation bottleneck (~76us on ACT in v1)
  * depth-2 software pipelining of the PE stream (QK(g) ... AV(g-2)) so
    the exp latency hides behind matmul work
  * output evacuation PSUM->SBUF alternates ACT/DVE (DMA can't read PSUM)
"""

import math

import numpy as np
import ml_dtypes

import bass_rust
import concourse.bass as bass
import concourse.mybir as mybir
import concourse.tile as tile
from concourse.bass_utils import run_bass_kernel_spmd

B, L, S, H, E, D = 2, 2048, 2048, 16, 64, 64
NCORES = 8
BH = B * H                      # 32 (b,h) pairs
BH_PER_CORE = BH // NCORES      # 4
SCALE = 1.0 / math.sqrt(E)
APRIME = 128.0 / math.log(2.0)  # exp(x) = 2^(a'*x / 128); folded into q/delta
B0 = 16248.65                   # Schraudolph bf16 bias, gmean-centered
LOG2_OVER_128 = math.log(2.0) / 128.0

F32 = mybir.dt.float32
BF16 = mybir.dt.bfloat16
I16 = mybir.dt.int16

VP_COLS = (S // 128) * (D + 1)  # 1040
NP_BF16 = ml_dtypes.bfloat16


class _SplitDrainTileContext(tile.TileContext):
    """This walrus build rejects instructions carrying more than one sem
    wait; the kernel-tail drain aggregates one wait per active processor.
    Split them across a chain of drains on the same engine."""

    def _drain_and_barrier(self, tick_clock, wait_clock):
        nc = self.nc
        drain_inst = nc.sync.drain()
        wait_clock.add_sem_waits(
            drain_inst.ins, bass_rust.ScopedClock({None: tick_clock.global_clock})
        )
        si = drain_inst.ins.sync_info
        waits = list(si.on_wait) if si is not None and si.on_wait else []
        if len(waits) > 1:
            si.on_wait = waits[:1]
            # spread the remaining waits across all engines' drains so they
            # resolve in parallel (each engine runs its drains serially);
            # the barrier below joins them.
            engines = [nc.sync, nc.scalar, nc.vector, nc.tensor, nc.gpsimd]
            for wi, w in enumerate(waits[1:]):
                d2 = engines[wi % len(engines)].drain()
                d2.ins.sync_info = bass_rust.SyncInfo(on_wait=[w], on_update=[])
        nc.all_engine_barrier()
        popped = nc._tile_sem_poison_stack.pop()
        assert popped is self._sem_poison
        nc.clear_and_free_semaphores(list(self.sems.allocated().values()))
        nc.all_engine_barrier()


def _legalize_waits(nc, max_waits=1):
    """This walrus build rejects instructions with more than `max_waits`
    sem waits. Spill extras onto same-engine NoOps inserted just before
    the offending instruction (same-engine program order preserves the
    wait semantics)."""
    for f in nc.m.functions:
        for bb in f.blocks:
            insts = bb.instructions
            for idx in range(len(insts) - 1, -1, -1):
                inst = insts[idx]
                si = getattr(inst, "sync_info", None)
                if si is None or not si.on_wait:
                    continue
                ow = list(si.on_wait)
                sem = [w for w in ow if w.sync_type == "semaphore"]
                other = [w for w in ow if w.sync_type != "semaphore"]
                budget = max(0, max_waits - len(other))
                if len(sem) <= budget:
                    continue
                keep, spill = sem[:budget], sem[budget:]
                si.on_wait = other + keep
                for w in reversed(spill):
                    n = mybir.InstNoOp(name=f"W-{nc.next_id()}", ins=[], outs=[])
                    n.engine = inst.engine
                    n.sync_info = bass_rust.SyncInfo(on_wait=[w], on_update=[])
                    nc.register_instruction(n, overwrite=True)
                    insts.insert(idx, n)


def _core_groups():
    """Static schedule: the packed causal piece/group structure for one core.

    Per (bh, qr) the valid strips are j = 0..4qr+3 with piece widths
    512 (j <= 4qr), 384, 256, 128. Pieces pack contiguously into PSUM
    group tiles (<= 1024 cols, never crossing a 512-col bank boundary):
      pairs  [512, 512] x 2qr   (alternating ACT/DVE)
      A      [512, 384]         (ACT)
      B      [256, 128]         (DVE)
    Each piece: (j, off, ln); derived rel = max(0,128j-l0), diag = j>=4qr.
    """
    groups = []
    for i in range(BH_PER_CORE):
        for qr in range(4):
            qgroups = []
            for t in range(2 * qr):
                qgroups.append(dict(
                    engine='act' if t % 2 == 0 else 'dve',
                    pieces=[(2 * t, 0, 512), (2 * t + 1, 512, 512)]))
            qgroups.append(dict(
                engine='act', pieces=[(4 * qr, 0, 512), (4 * qr + 1, 512, 384)]))
            qgroups.append(dict(
                engine='dve', pieces=[(4 * qr + 2, 0, 256), (4 * qr + 3, 256, 128)]))
            for gi, g in enumerate(qgroups):
                g.update(i=i, qr=qr, last=(gi == len(qgroups) - 1))
                groups.append(g)
    return groups


def _build_program():
    nc = bass.Bass("TRN2", target_bir_lowering=False, debug=False)
    # q: rows 0:64 = (a'*scale*tau)*q^T, row 64 = 1.0
    # k: rows 0:64 = k^T,                row 64 = (a'*scale)*delta
    # v: col 65*j + c = V'[128j+p, c], V' = [v | ones]
    q_d = nc.declare_dram_parameter("q", [BH_PER_CORE, E + 1, L], BF16, isOutput=False)
    k_d = nc.declare_dram_parameter("k", [BH_PER_CORE, E + 1, S], BF16, isOutput=False)
    v_d = nc.declare_dram_parameter("v", [BH_PER_CORE, 128, VP_COLS], BF16, isOutput=False)
    # output stays transposed: [bh, quarter, d, l_rel]; row d == D is the
    # softmax denominator; the host divides + transposes.
    o_d = nc.declare_dram_parameter("o", [BH_PER_CORE, 4, D + 1, 512], F32, isOutput=True)

    groups = _core_groups()

    with _SplitDrainTileContext(nc) as tc:
        with (
            tc.tile_pool(name="qin", bufs=1) as q_pool,
            tc.tile_pool(name="p", bufs=4) as p_pool,
            tc.tile_pool(name="osb", bufs=3) as osb_pool,
            tc.tile_pool(name="xt_ps", bufs=3, space="PSUM") as xt_pool,
            tc.tile_pool(name="out_ps", bufs=2, space="PSUM") as ot_pool,
        ):
            qts, kts, vts = [], [], []
            for i in range(BH_PER_CORE):
                qts.append(q_pool.tile([E + 1, L], BF16, name=f"q{i}", tag=f"q{i}"))
                kts.append(q_pool.tile([E + 1, S], BF16, name=f"k{i}", tag=f"k{i}"))
                vts.append(q_pool.tile([128, VP_COLS], BF16, name=f"v{i}", tag=f"v{i}"))
            # bh0's first pieces are on the critical path: issue their DMAs
            # from three different engines in parallel (HWDGE issue is
            # ~0.6us each; SP alone would serialize 0.6us per dma_start)
            # and partition-split them across two queues each.
            nc.scalar.dma_start(out=qts[0][0:33, 0:512], in_=q_d[0, 0:33, 0:512])
            nc.scalar.dma_start(out=qts[0][33:65, 0:512], in_=q_d[0, 33:65, 0:512])
            nc.sync.dma_start(out=kts[0][0:33, 0:512], in_=k_d[0, 0:33, 0:512])
            nc.sync.dma_start(out=kts[0][33:65, 0:512], in_=k_d[0, 33:65, 0:512])
            nc.sync.dma_start(out=vts[0][0:64, 0:260], in_=v_d[0, 0:64, 0:260])
            nc.sync.dma_start(out=vts[0][64:128, 0:260], in_=v_d[0, 64:128, 0:260])
            nc.sync.dma_start(out=qts[0][:, 512:1024], in_=q_d[0, :, 512:1024])
            nc.sync.dma_start(out=kts[0][:, 512:1024], in_=k_d[0, :, 512:1024])
            nc.sync.dma_start(out=vts[0][:, 260:520], in_=v_d[0, :, 260:520])
            nc.sync.dma_start(out=qts[0][:, 1024:2048], in_=q_d[0, :, 1024:2048])
            nc.sync.dma_start(out=kts[0][:, 1024:2048], in_=k_d[0, :, 1024:2048])
            nc.sync.dma_start(out=vts[0][:, 520:1040], in_=v_d[0, :, 520:1040])
            for i in range(1, BH_PER_CORE):
                nc.sync.dma_start(out=qts[i][:, 0:1024], in_=q_d[i, :, 0:1024])
                nc.sync.dma_start(out=kts[i][:, 0:1024], in_=k_d[i, :, 0:1024])
                nc.sync.dma_start(out=vts[i][:, 0:520], in_=v_d[i, :, 0:520])
                nc.sync.dma_start(out=qts[i][:, 1024:2048], in_=q_d[i, :, 1024:2048])
                nc.sync.dma_start(out=kts[i][:, 1024:2048], in_=k_d[i, :, 1024:2048])
                nc.sync.dma_start(out=vts[i][:, 520:1040], in_=v_d[i, :, 520:1040])

            ots = {}          # (i, qr) -> out PSUM tile
            evac_n = [0]

            def emit_av(g):
                i, qr = g['i'], g['qr']
                l0 = 512 * qr
                if (i, qr) not in ots:
                    ots[(i, qr)] = ot_pool.tile([D + 1, 512], F32, name="ot", tag="ot")
                ot = ots[(i, qr)]
                for (j, off, ln) in g['pieces']:
                    rel = max(0, 128 * j - l0)
                    nc.tensor.matmul(
                        ot[:, rel:rel + ln],
                        lhsT=vts[i][:, (D + 1) * j:(D + 1) * (j + 1)],
                        rhs=g['p'][:, off:off + ln].bitcast(BF16),
                        start=(j == 0), stop=(j == 4 * qr + 3),
                    )
                if g['last']:
                    ot_sb = osb_pool.tile([D + 1, 512], F32)
                    evac_n[0] += 1
                    # outputs go out the Activation engine's DMA queue (the
                    # SP queue carries the input stream; each engine owns
                    # ONE hw queue). The final output's evac is split
                    # ACT/DVE and its DMA across both queues so the
                    # post-last-matmul serial chain is as short as possible.
                    if evac_n[0] == 4 * BH_PER_CORE:
                        nc.scalar.activation(
                            ot_sb[:, 0:256], ot[:, 0:256],
                            mybir.ActivationFunctionType.Copy)
                        nc.vector.tensor_copy(ot_sb[:, 256:512], ot[:, 256:512])
                        nc.scalar.dma_start(
                            out=o_d[i, qr, :, 0:256], in_=ot_sb[:, 0:256])
                        nc.sync.dma_start(
                            out=o_d[i, qr, :, 256:512], in_=ot_sb[:, 256:512])
                    else:
                        if evac_n[0] % 2 == 1:
                            nc.scalar.activation(
                                ot_sb, ot, mybir.ActivationFunctionType.Copy)
                        else:
                            nc.vector.tensor_copy(ot_sb, ot)
                        nc.scalar.dma_start(out=o_d[i, qr], in_=ot_sb)

            for gidx, g in enumerate(groups):
                i, qr = g['i'], g['qr']
                l0 = 512 * qr
                width = sum(ln for (_, _, ln) in g['pieces'])
                xt = xt_pool.tile([128, 1024], F32)
                for (j, off, ln) in g['pieces']:
                    ls = max(l0, 128 * j)
                    nc.tensor.matmul(
                        xt[:, off:off + ln],
                        lhsT=kts[i][:, 128 * j:128 * (j + 1)],
                        rhs=qts[i][:, ls:ls + ln],
                        start=True, stop=True,
                    )
                p = p_pool.tile([128, 1024], BF16)
                g['p'] = p
                if g['engine'] == 'act':
                    nc.scalar.activation(
                        p[:, 0:width], xt[:, 0:width],
                        mybir.ActivationFunctionType.Exp,
                        scale=LOG2_OVER_128,
                    )
                else:
                    nc.vector.tensor_scalar(
                        p[:, 0:width].bitcast(I16), xt[:, 0:width],
                        float(B0), None, mybir.AluOpType.add,
                    )
                for (j, off, ln) in g['pieces']:
                    if 128 * j >= l0:
                        # diagonal block: zero p where s > l
                        # (keep where (l - s) >= 0)
                        nc.gpsimd.affine_select(
                            out=p[:, off:off + 128],
                            in_=p[:, off:off + 128],
                            compare_op=mybir.AluOpType.is_ge, fill=0.0,
                            base=0, channel_multiplier=-1,
                            pattern=[[1, 128]],
                        )
                # depth-2 software pipeline: PE stream is QK(g), AV(g-2)
                if gidx >= 2:
                    emit_av(groups[gidx - 2])
            emit_av(groups[-2])
            emit_av(groups[-1])
    _legalize_waits(nc)
    return nc


_PROGRAM = None


def _get_program():
    global _PROGRAM
    if _PROGRAM is None:
        _PROGRAM = _build_program()
    return _PROGRAM


def _prepare_inputs(q, k, v, tau, delta):
    """Pack full inputs into per-core bf16 device layouts."""
    qs = (q.astype(np.float64)
          * (SCALE * APRIME * tau.astype(np.float64))[:, 0, None, None, None]
          ).astype(np.float32)
    qt = qs.transpose(0, 2, 3, 1).reshape(BH, E, L)
    kt = k.transpose(0, 2, 3, 1).reshape(BH, E, S)
    dsc = (SCALE * APRIME * delta).astype(np.float32)    # [B, S]
    xq = np.concatenate([qt, np.ones((BH, 1, L), np.float32)], 1)
    xk = np.concatenate([kt, np.repeat(dsc, H, axis=0)[:, None, :]], 1)
    vt = v.transpose(0, 2, 1, 3).reshape(BH, S, D)
    vp = np.concatenate([vt, np.ones((BH, S, 1), np.float32)], axis=2)
    vp = (vp.reshape(BH, S // 128, 128, D + 1)
          .transpose(0, 2, 1, 3).reshape(BH, 128, VP_COLS))
    return (np.ascontiguousarray(xq).astype(NP_BF16),
            np.ascontiguousarray(xk).astype(NP_BF16),
            np.ascontiguousarray(vp).astype(NP_BF16))


def _numpy_fallback(q, k, v, att_mask, tau, delta):
    out = np.empty((B, L, H, D), np.float32)
    mask = att_mask[:, 0]  # [B, L, S]
    for b in range(B):
        for h in range(H):
            s = (q[b, :, h, :] @ k[b, :, h, :].T) * tau[b, 0] + delta[b][None, :]
            s = np.where(mask[b], -1e9, s).astype(np.float32)
            s = SCALE * s
            s = s - s.max(axis=-1, keepdims=True)
            e = np.exp(s)
            a = e / e.sum(axis=-1, keepdims=True)
            out[b, :, h, :] = a @ v[b, :, h, :]
    return out


def kernel(q, k, v, att_mask, tau, delta):
    q = np.asarray(q, np.float32)
    k = np.asarray(k, np.float32)
    v = np.asarray(v, np.float32)
    tau = np.asarray(tau, np.float32)
    delta = np.asarray(delta, np.float32)
    att_mask = np.asarray(att_mask)

    causal = np.triu(np.ones((L, S), bool), k=1)
    if not all(np.array_equal(att_mask[b, 0], causal) for b in range(B)):
        return _numpy_fallback(q, k, v, att_mask, tau, delta)

    xq, xk, vp = _prepare_inputs(q, k, v, tau, delta)
    nc = _get_program()
    in_maps = [
        {
            "q": np.ascontiguousarray(xq[c * BH_PER_CORE:(c + 1) * BH_PER_CORE]),
            "k": np.ascontiguousarray(xk[c * BH_PER_CORE:(c + 1) * BH_PER_CORE]),
            "v": np.ascontiguousarray(vp[c * BH_PER_CORE:(c + 1) * BH_PER_CORE]),
        }
        for c in range(NCORES)
    ]
    res = run_bass_kernel_spmd(nc, in_maps, list(range(NCORES))).results

    out = np.empty((B, L, H, D), np.float32)
    for c in range(NCORES):
        o = res[c]["o"]  # [4, 4, D+1, 512]: raw numerators + denominator row
        norm = o[:, :, 0:D, :] / o[:, :, D:D + 1, :]
        for i in range(BH_PER_CORE):
            bh = c * BH_PER_CORE + i
            out[bh // H, :, bh % H, :] = norm[i].transpose(0, 2, 1).reshape(L, D)
    return out
